# revision 6
# baseline (speedup 1.0000x reference)
"""Trainium2 Bass kernel for the GRU caption model.

Computes: h0 = feat @ W_hp.T + b_hp; 200-step GRU with constant hidden-proj
gate pre-activations; logits = outs @ W_out.T + b_out -> [B, V, T].

Strategy: every core runs the (tiny, latency-bound) GRU redundantly; the
vocab dimension of W_out is sharded 8 ways; each core emits its own
[B, 3840, T] logits slice which the host concatenates.

All on-chip compute uses a transposed [feature-on-partitions, batch-free]
layout so the recurrent state feeds the next step's matmul directly.
"""

import numpy as np
import ml_dtypes

import concourse.bass as bass
import concourse.mybir as mybir
import concourse.tile as tile
from concourse import bacc
from concourse.bass_utils import run_bass_kernel_spmd

F32 = mybir.dt.float32
F32R = mybir.dt.float32r
BF16 = mybir.dt.bfloat16
AF = mybir.ActivationFunctionType
ALU = mybir.AluOpType

VOCAB = 30522
HID = 512
FEAT = 2048
STEPS = 200
BATCH = 32
SOS = 101
NCORES = 8
P = 128
KO = HID // P          # 4 h-chunks
GM = 3 * HID // P      # 12 gate row-groups (r: 0-3, z: 4-7, n: 8-11)
KF = FEAT // P         # 16 feat chunks
VPAD = 3840            # per-core padded vocab rows = 30 * 128
MT = VPAD // P         # 30 vocab tiles per core
TBLOCKS = [(0, 64), (64, 128), (128, 200)]  # proj t-blocks (N = 4*len >= 256)

LAST_RESULTS = None  # test harness introspection


def _r(ap):
    """Reinterpret an fp32 AP as float32r for full-rate PE streaming."""
    return ap.bitcast(F32R)


def build():
    nc = bacc.Bacc("TRN2", target_bir_lowering=False, debug=False)

    featT = nc.dram_tensor("featT", [FEAT, BATCH], F32, kind="ExternalInput")
    WhpT = nc.dram_tensor("WhpT", [FEAT, HID], F32, kind="ExternalInput")
    WihT = nc.dram_tensor("WihT", [HID, 3 * HID], BF16, kind="ExternalInput")
    WhhT = nc.dram_tensor("WhhT", [HID, 3 * HID], F32, kind="ExternalInput")
    b_ih = nc.dram_tensor("b_ih", [3 * HID], F32, kind="ExternalInput")
    b_hh = nc.dram_tensor("b_hh", [3 * HID], F32, kind="ExternalInput")
    b_hp = nc.dram_tensor("b_hp", [HID], F32, kind="ExternalInput")
    x0T = nc.dram_tensor("x0T", [HID, BATCH], BF16, kind="ExternalInput")
    WoutT = nc.dram_tensor("WoutT", [HID, VPAD], F32R, kind="ExternalInput")
    b_out = nc.dram_tensor("b_out", [VPAD], F32, kind="ExternalInput")
    OUT = nc.dram_tensor("OUT", [BATCH, VPAD, STEPS], F32, kind="ExternalOutput")

    with tile.TileContext(nc) as tc:
        with (
            tc.tile_pool(name="const", bufs=1) as const,
            tc.tile_pool(name="stream", bufs=3) as stream,
            tc.tile_pool(name="step", bufs=3) as sp,
            tc.tile_pool(name="hb", bufs=3) as hb,
            tc.tile_pool(name="outp", bufs=6) as outp,
            tc.tile_pool(name="psg", bufs=2, space="PSUM") as psg,
            tc.tile_pool(name="psp", bufs=4, space="PSUM") as psp,
        ):
            # ---- constants into SBUF ----
            wih = const.tile([P, KO, GM, P], BF16, tag="wih")
            nc.sync.dma_start(
                wih[:], WihT.rearrange("(k p) (m c) -> p k m c", p=P, c=P)
            )
            featT_sb = const.tile([P, KF, BATCH], F32, tag="featsb")
            nc.sync.dma_start(featT_sb[:], featT.rearrange("(k p) b -> p k b", p=P))
            bih_sb = const.tile([P, GM], F32, tag="bih")
            nc.sync.dma_start(bih_sb[:], b_ih.rearrange("(m p) -> p m", p=P))
            bhh_sb = const.tile([P, GM], F32, tag="bhh")
            nc.sync.dma_start(bhh_sb[:], b_hh.rearrange("(m p) -> p m", p=P))
            bhp_sb = const.tile([P, KO], F32, tag="bhp")
            nc.sync.dma_start(bhp_sb[:], b_hp.rearrange("(m p) -> p m", p=P))
            bout_sb = const.tile([P, MT], F32, tag="bout")
            nc.sync.dma_start(bout_sb[:], b_out.rearrange("(m p) -> p m", p=P))

            WhpT_r = WhpT.rearrange("(k p) h -> p k h", p=P)
            WhhT_r = WhhT.rearrange("(k p) g -> p k g", p=P)
            WoutT_r = WoutT.rearrange("(k p) v -> p k v", p=P)

            # ---- h0 = feat @ W_hp.T + b_hp (fp32, exact) ----
            ps_h = psg.tile([P, GM, BATCH], F32, tag="gates")
            for ko in range(KO):
                for kf in range(KF):
                    wt = stream.tile([P, P], F32, tag="whp")
                    nc.sync.dma_start(wt[:], WhpT_r[:, kf, ko * P:(ko + 1) * P])
                    nc.tensor.matmul(
                        ps_h[:, ko, :], wt[:], featT_sb[:, kf, :],
                        start=(kf == 0), stop=(kf == KF - 1),
                    )
            h0T = const.tile([P, KO, BATCH], F32, tag="h0T")
            for ko in range(KO):
                nc.scalar.activation(
                    h0T[:, ko, :], ps_h[:, ko, :], AF.Identity,
                    bias=bhp_sb[:, ko, None], scale=1.0,
                )
            h0_half = const.tile([P, KO, BATCH], F32, tag="h0h")
            nc.scalar.mul(h0_half[:], h0T[:], 0.5)

            # ---- gh = h0 @ W_hh.T + b_hh (fp32, exact; step-invariant) ----
            ps_g = psg.tile([P, GM, BATCH], F32, tag="gates")
            for m in range(GM):
                for k in range(KO):
                    wt = stream.tile([P, P], F32, tag="whh")
                    nc.sync.dma_start(wt[:], WhhT_r[:, k, m * P:(m + 1) * P])
                    nc.tensor.matmul(
                        ps_g[:, m, :], wt[:], h0T[:, k, :],
                        start=(k == 0), stop=(k == KO - 1),
                    )
            ghT = const.tile([P, GM, BATCH], F32, tag="ghT")
            for m in range(GM):
                nc.scalar.activation(
                    ghT[:, m, :], ps_g[:, m, :], AF.Identity,
                    bias=bhh_sb[:, m, None], scale=1.0,
                )
            # C_rz = gh_rz + b_ih_rz ; hn2 = 0.5*gh_n ; E_n = hn2 + b_ih_n
            C_rz = const.tile([P, 8, BATCH], F32, tag="Crz")
            nc.vector.tensor_add(
                C_rz[:], ghT[:, 0:8, :],
                bih_sb[:, 0:8, None].to_broadcast((P, 8, BATCH)),
            )
            hn2 = const.tile([P, KO, BATCH], F32, tag="hn2")
            nc.scalar.mul(hn2[:], ghT[:, 8:12, :], 0.5)
            E_n = const.tile([P, KO, BATCH], F32, tag="En")
            nc.vector.tensor_add(
                E_n[:], hn2[:],
                bih_sb[:, 8:12, None].to_broadcast((P, KO, BATCH)),
            )

            # resT blocks: col = b*bsize + (t - t0), per h-chunk ko
            resT = []
            for j, (t0, t1) in enumerate(TBLOCKS):
                bs = t1 - t0
                rt = const.tile(
                    [P, KO, BATCH, bs], F32R, tag=f"resT{j}", name=f"resT{j}"
                )
                resT.append(rt)

            prev = hb.tile([P, KO, BATCH], BF16, tag="hb")
            nc.sync.dma_start(prev[:], x0T.rearrange("(k p) b -> p k b", p=P))

            def proj_block(j):
                t0, t1 = TBLOCKS[j]
                bs = t1 - t0
                N = 4 * bs
                for m in range(MT):
                    wt = stream.tile([P, KO, P], F32R, tag="wout")
                    nc.sync.dma_start(wt[:], WoutT_r[:, :, m * P:(m + 1) * P])
                    for g in range(BATCH // 4):
                        ps_full = psp.tile([P, 288], F32, tag="pp", name="pp")
                        ps = ps_full[:, :N]
                        for k in range(KO):
                            nc.tensor.matmul(
                                ps,
                                wt[:, k, :],
                                resT[j][:, k, 4 * g:4 * g + 4, :],
                                start=(k == 0), stop=(k == KO - 1),
                            )
                        ob_full = outp.tile([P, 288], F32, tag="ob", name="ob")
                        ob = ob_full[:, :N]
                        if (m + g) % 2 == 0:
                            nc.scalar.activation(
                                ob, ps, AF.Identity,
                                bias=bout_sb[:, m, None], scale=1.0,
                            )
                        else:
                            nc.vector.tensor_scalar_add(ob, ps, bout_sb[:, m, None])
                        dst = OUT[
                            4 * g:4 * g + 4, m * P:(m + 1) * P, t0:t1
                        ].rearrange("b v t -> v b t")
                        nc.sync.dma_start(dst, ob.rearrange("p (b t) -> p b t", b=4))

            # ---- GRU steps ----
            mm_order = [8, 9, 10, 11] + list(range(8))  # n-gates first
            for t in range(STEPS):
                ps = psg.tile([P, GM, BATCH], F32, tag="gates")
                for m in mm_order:
                    for k in range(KO):
                        nc.tensor.matmul(
                            ps[:, m, :], wih[:, k, m, :], prev[:, k, :],
                            start=(k == 0), stop=(k == KO - 1),
                        )
                s_rz = sp.tile([P, 8, BATCH], F32, tag="srz")
                nc.vector.tensor_add(s_rz[:], ps[:, 0:8, :], C_rz[:])
                t_rz = sp.tile([P, 8, BATCH], F32, tag="trz")
                nc.scalar.activation(t_rz[:], s_rz[:], AF.Tanh, scale=0.5)
                a = sp.tile([P, KO, BATCH], F32, tag="a")
                nc.vector.tensor_mul(a[:], t_rz[:, 0:4, :], hn2[:])
                sn1 = sp.tile([P, KO, BATCH], F32, tag="sn1")
                nc.vector.tensor_add(sn1[:], ps[:, 8:12, :], E_n[:])
                sn2 = sp.tile([P, KO, BATCH], F32, tag="sn2")
                nc.vector.tensor_add(sn2[:], sn1[:], a[:])
                n = sp.tile([P, KO, BATCH], F32, tag="n")
                nc.scalar.activation(n[:], sn2[:], AF.Tanh, scale=1.0)
                q = sp.tile([P, KO, BATCH], F32, tag="q")
                nc.vector.tensor_sub(q[:], h0T[:], n[:])
                w2 = sp.tile([P, KO, BATCH], F32, tag="w2")
                nc.vector.scalar_tensor_tensor(
                    w2[:], t_rz[:, 4:8, :], 0.5, q[:], ALU.mult, ALU.mult
                )
                p2 = sp.tile([P, KO, BATCH], F32, tag="p2")
                nc.vector.scalar_tensor_tensor(
                    p2[:], n[:], 0.5, h0_half[:], ALU.mult, ALU.add
                )
                hT = sp.tile([P, KO, BATCH], F32, tag="hT")
                nc.vector.tensor_add(hT[:], w2[:], p2[:])
                nxt = hb.tile([P, KO, BATCH], BF16, tag="hb")
                nc.gpsimd.tensor_copy(nxt[:], hT[:])
                j = 0 if t < 64 else (1 if t < 128 else 2)
                t0 = TBLOCKS[j][0]
                nc.scalar.copy(resT[j][:, :, :, t - t0], hT[:])
                prev = nxt
                if t == TBLOCKS[j][1] - 1:
                    proj_block(j)

    nc.compile()
    return nc


def _shard_inputs(feat, W_hp, b_hp, W_ih, W_hh, b_ih, b_hh, embed, W_out, b_out):
    bf = ml_dtypes.bfloat16
    featT = np.ascontiguousarray(feat.T, dtype=np.float32)
    WhpT = np.ascontiguousarray(W_hp.T, dtype=np.float32)
    WihT = np.ascontiguousarray(W_ih.T).astype(bf)
    WhhT = np.ascontiguousarray(W_hh.T, dtype=np.float32)
    x0T = np.ascontiguousarray(
        np.repeat(np.asarray(embed)[SOS][:, None], BATCH, axis=1)
    ).astype(bf)
    Wo = np.zeros((NCORES * VPAD, HID), np.float32)
    Wo[:VOCAB] = W_out
    bo = np.zeros((NCORES * VPAD,), np.float32)
    bo[:VOCAB] = b_out
    common = dict(
        featT=featT, WhpT=WhpT, WihT=WihT, WhhT=WhhT,
        b_ih=np.asarray(b_ih, np.float32), b_hh=np.asarray(b_hh, np.float32),
        b_hp=np.asarray(b_hp, np.float32), x0T=x0T,
    )
    in_maps = []
    for c in range(NCORES):
        sl = slice(c * VPAD, (c + 1) * VPAD)
        m = dict(common)
        m["WoutT"] = np.ascontiguousarray(Wo[sl].T)
        m["b_out"] = bo[sl].copy()
        in_maps.append(m)
    return in_maps


def kernel(**inputs):
    global LAST_RESULTS
    args = {k: np.asarray(v) for k, v in inputs.items()}
    in_maps = _shard_inputs(
        args["feat"], args["W_hp"], args["b_hp"], args["W_ih"], args["W_hh"],
        args["b_ih"], args["b_hh"], args["embed"], args["W_out"], args["b_out"],
    )
    nc = build()
    res = run_bass_kernel_spmd(nc, in_maps, core_ids=list(range(NCORES)))
    LAST_RESULTS = res
    out = np.concatenate([r["OUT"] for r in res.results], axis=1)[:, :VOCAB, :]
    return np.ascontiguousarray(out, dtype=np.float32)


# revision 16
# speedup vs baseline: 39845.7556x; 39845.7556x over previous
"""Trainium2 Bass kernel for the GRU caption model.

Computes: h0 = feat @ W_hp.T + b_hp; 200-step GRU with constant hidden-proj
gate pre-activations; logits = outs @ W_out.T + b_out -> [B, V, T].

Strategy: every core runs the (tiny, latency-bound) GRU redundantly; the
vocab dimension of W_out is sharded 8 ways; each core emits its own
[B, 3840, T] logits slice which the host concatenates.

All on-chip compute uses a transposed [feature-on-partitions, batch-free]
layout so the recurrent state feeds the next step's matmul directly.
"""

import numpy as np
import ml_dtypes

import concourse.bass as bass
import concourse.mybir as mybir
import concourse.tile as tile
from concourse import bacc
from concourse.bass_utils import run_bass_kernel_spmd

F32 = mybir.dt.float32
F32R = mybir.dt.float32r
BF16 = mybir.dt.bfloat16
AF = mybir.ActivationFunctionType
ALU = mybir.AluOpType

VOCAB = 30522
HID = 512
FEAT = 2048
STEPS = 200
BATCH = 32
SOS = 101
NCORES = 8
P = 128
KO = HID // P          # 4 h-chunks
GM = 3 * HID // P      # 12 gate row-groups (r: 0-3, z: 4-7, n: 8-11)
KF = FEAT // P         # 16 feat chunks
VPAD = 3840            # per-core padded vocab rows = 30 * 128
MT = VPAD // P         # 30 vocab tiles per core
TBLOCKS = [(0, 64), (64, 128), (128, 200)]  # proj t-blocks

LAST_RESULTS = None  # test harness introspection
EMIT_GRU = True    # variant switch (sim experiments)
EMIT_PROJ = True   # variant switch (sim experiments)
PROJ_MODE = 2      # 0 = matmuls only, 1 = +copies, 2 = +DMA (sim experiments)


def _r(ap):
    """Reinterpret an fp32 AP as float32r for full-rate PE streaming."""
    return ap.bitcast(F32R)


def build():
    nc = bacc.Bacc("TRN2", target_bir_lowering=False, debug=False)

    featT = nc.dram_tensor("featT", [FEAT, BATCH], F32, kind="ExternalInput")
    WhpT = nc.dram_tensor("WhpT", [FEAT, HID], F32, kind="ExternalInput")
    WihT = nc.dram_tensor("WihT", [HID, 3 * HID], BF16, kind="ExternalInput")
    WhhT = nc.dram_tensor("WhhT", [HID, 3 * HID], F32, kind="ExternalInput")
    b_ih = nc.dram_tensor("b_ih", [3 * HID], F32, kind="ExternalInput")
    b_hh = nc.dram_tensor("b_hh", [3 * HID], F32, kind="ExternalInput")
    b_hp = nc.dram_tensor("b_hp", [HID], F32, kind="ExternalInput")
    x0T = nc.dram_tensor("x0T", [HID, BATCH], BF16, kind="ExternalInput")
    WoutT = nc.dram_tensor("WoutT", [HID, VPAD], F32R, kind="ExternalInput")
    b_out = nc.dram_tensor("b_out", [VPAD], F32, kind="ExternalInput")
    OUT = nc.dram_tensor("OUT", [BATCH, VPAD, STEPS], F32, kind="ExternalOutput")

    with tile.TileContext(nc) as tc:
        with (
            tc.tile_pool(name="const", bufs=1) as const,
            tc.tile_pool(name="stream", bufs=3) as stream,
            tc.tile_pool(name="step", bufs=4) as sp,
            tc.tile_pool(name="hb", bufs=4) as hb,
            tc.tile_pool(name="outp", bufs=6) as outp,
            tc.tile_pool(name="psg", bufs=3, space="PSUM") as psg,
            tc.tile_pool(name="psp", bufs=4, space="PSUM") as psp,
        ):
            # ---- constants into SBUF ----
            wih = const.tile([P, KO, GM, P], BF16, tag="wih")
            nc.sync.dma_start(
                wih[:], WihT.rearrange("(k p) (m c) -> p k m c", p=P, c=P)
            )
            featT_sb = const.tile([P, KF, BATCH], F32, tag="featsb")
            nc.sync.dma_start(featT_sb[:], featT.rearrange("(k p) b -> p k b", p=P))
            bih_sb = const.tile([P, GM], F32, tag="bih")
            nc.sync.dma_start(bih_sb[:], b_ih.rearrange("(m p) -> p m", p=P))
            bhh_sb = const.tile([P, GM], F32, tag="bhh")
            nc.sync.dma_start(bhh_sb[:], b_hh.rearrange("(m p) -> p m", p=P))
            bhp_sb = const.tile([P, KO], F32, tag="bhp")
            nc.sync.dma_start(bhp_sb[:], b_hp.rearrange("(m p) -> p m", p=P))
            bout_sb = const.tile([P, MT], F32, tag="bout")
            nc.sync.dma_start(bout_sb[:], b_out.rearrange("(m p) -> p m", p=P))

            WhpT_r = WhpT.rearrange("(k p) h -> p k h", p=P)
            WhhT_r = WhhT.rearrange("(k p) g -> p k g", p=P)
            WoutT_r = WoutT.rearrange("(k p) v -> p k v", p=P)

            # ---- h0 = feat @ W_hp.T + b_hp (fp32, exact) ----
            ps_h = psg.tile([P, GM, BATCH], F32, tag="gates")
            for ko in range(KO):
                for kf in range(KF):
                    wt = stream.tile([P, P], F32, tag="whp")
                    nc.sync.dma_start(wt[:], WhpT_r[:, kf, ko * P:(ko + 1) * P])
                    nc.tensor.matmul(
                        ps_h[:, ko, :], wt[:], featT_sb[:, kf, :],
                        start=(kf == 0), stop=(kf == KF - 1),
                    )
            h0T = const.tile([P, KO, BATCH], F32, tag="h0T")
            for ko in range(KO):
                nc.scalar.activation(
                    h0T[:, ko, :], ps_h[:, ko, :], AF.Identity,
                    bias=bhp_sb[:, ko, None], scale=1.0,
                )
            h0_half = const.tile([P, KO, BATCH], F32, tag="h0h")
            nc.scalar.mul(h0_half[:], h0T[:], 0.5)

            # ---- gh = h0 @ W_hh.T + b_hh (fp32, exact; step-invariant) ----
            ps_g = psg.tile([P, GM, BATCH], F32, tag="gates")
            for m in range(GM):
                for k in range(KO):
                    wt = stream.tile([P, P], F32, tag="whh")
                    nc.sync.dma_start(wt[:], WhhT_r[:, k, m * P:(m + 1) * P])
                    nc.tensor.matmul(
                        ps_g[:, m, :], wt[:], h0T[:, k, :],
                        start=(k == 0), stop=(k == KO - 1),
                    )
            ghT = const.tile([P, GM, BATCH], F32, tag="ghT")
            for m in range(GM):
                nc.scalar.activation(
                    ghT[:, m, :], ps_g[:, m, :], AF.Identity,
                    bias=bhh_sb[:, m, None], scale=1.0,
                )
            # C_rz = gh_rz + b_ih_rz ; hn2 = 0.5*gh_n ; E_n = hn2 + b_ih_n
            C_rz = const.tile([P, 8, BATCH], F32, tag="Crz")
            nc.vector.tensor_add(
                C_rz[:], ghT[:, 0:8, :],
                bih_sb[:, 0:8, None].to_broadcast((P, 8, BATCH)),
            )
            hn2 = const.tile([P, KO, BATCH], F32, tag="hn2")
            nc.scalar.mul(hn2[:], ghT[:, 8:12, :], 0.5)
            E_n = const.tile([P, KO, BATCH], F32, tag="En")
            nc.vector.tensor_add(
                E_n[:], hn2[:],
                bih_sb[:, 8:12, None].to_broadcast((P, KO, BATCH)),
            )

            # resT blocks: col = b*bsize + (t - t0), per h-chunk ko
            resT = []
            for j, (t0, t1) in enumerate(TBLOCKS):
                bs = t1 - t0
                rt = const.tile(
                    [P, KO, BATCH, bs], F32R, tag=f"resT{j}", name=f"resT{j}"
                )
                resT.append(rt)

            prev = hb.tile([P, KO, BATCH], BF16, tag="hb")
            nc.sync.dma_start(prev[:], x0T.rearrange("(k p) b -> p k b", p=P))

            def proj_block(j):
                t0, t1 = TBLOCKS[j]
                bs = t1 - t0
                gb = 4
                N = gb * bs
                for m in range(MT):
                    wt = stream.tile([P, KO, P], F32R, tag="wout")
                    nc.sync.dma_start(wt[:], WoutT_r[:, :, m * P:(m + 1) * P])
                    for g in range(BATCH // gb):
                        ps_full = psp.tile([P, 288], F32, tag="pp", name="pp")
                        ps = ps_full[:, :N]
                        for k in range(KO):
                            nc.tensor.matmul(
                                ps,
                                wt[:, k, :],
                                resT[j][:, k, gb * g:gb * g + gb, :],
                                start=(k == 0), stop=(k == KO - 1),
                            )
                        if PROJ_MODE == 0:
                            continue
                        ob_full = outp.tile([P, 288], F32, tag="ob", name="ob")
                        ob = ob_full[:, :N]
                        if (m + g) % 2 == 0:
                            nc.scalar.activation(
                                ob, ps, AF.Identity,
                                bias=bout_sb[:, m, None], scale=1.0,
                            )
                        else:
                            nc.vector.tensor_scalar_add(ob, ps, bout_sb[:, m, None])
                        if PROJ_MODE >= 2:
                            dst = OUT[
                                gb * g:gb * g + gb, m * P:(m + 1) * P, t0:t1
                            ].rearrange("b v t -> v b t")
                            nc.sync.dma_start(
                                dst, ob.rearrange("p (b t) -> p b t", b=gb)
                            )

            # ---- GRU steps ----
            if not EMIT_GRU:
                for j in range(len(TBLOCKS)):
                    nc.vector.memset(resT[j][:], 0.25)
                    proj_block(j)
            mm_order = [8, 9, 10, 11] + list(range(8))  # n-gates first
            for t in range(STEPS if EMIT_GRU else 0):
                ps = psg.tile([P, GM, BATCH], F32, tag="gates")
                for m in mm_order:
                    for k in range(KO):
                        nc.tensor.matmul(
                            ps[:, m, :], wih[:, k, m, :], prev[:, k, :],
                            start=(k == 0), stop=(k == KO - 1),
                        )
                s_rz = sp.tile([P, 8, BATCH], F32, tag="srz")
                nc.vector.tensor_add(s_rz[:], ps[:, 0:8, :], C_rz[:])
                t_rz = sp.tile([P, 8, BATCH], F32, tag="trz")
                nc.scalar.activation(t_rz[:], s_rz[:], AF.Tanh, scale=0.5)
                a = sp.tile([P, KO, BATCH], F32, tag="a")
                nc.vector.tensor_mul(a[:], t_rz[:, 0:4, :], hn2[:])
                sn1 = sp.tile([P, KO, BATCH], F32, tag="sn1")
                nc.vector.tensor_add(sn1[:], ps[:, 8:12, :], E_n[:])
                sn2 = sp.tile([P, KO, BATCH], F32, tag="sn2")
                nc.vector.tensor_add(sn2[:], sn1[:], a[:])
                n = sp.tile([P, KO, BATCH], F32, tag="n")
                nc.scalar.activation(n[:], sn2[:], AF.Tanh, scale=1.0)
                q = sp.tile([P, KO, BATCH], F32, tag="q")
                nc.vector.tensor_sub(q[:], h0T[:], n[:])
                w2 = sp.tile([P, KO, BATCH], F32, tag="w2")
                nc.vector.scalar_tensor_tensor(
                    w2[:], t_rz[:, 4:8, :], 0.5, q[:], ALU.mult, ALU.mult
                )
                p2 = sp.tile([P, KO, BATCH], F32, tag="p2")
                nc.vector.scalar_tensor_tensor(
                    p2[:], n[:], 0.5, h0_half[:], ALU.mult, ALU.add
                )
                nxt = hb.tile([P, KO, BATCH], BF16, tag="hb")
                nc.vector.tensor_add(nxt[:], w2[:], p2[:])
                j = next(i for i, (a, b) in enumerate(TBLOCKS) if a <= t < b)
                t0 = TBLOCKS[j][0]
                nc.gpsimd.tensor_add(resT[j][:, :, :, t - t0], w2[:], p2[:])
                prev = nxt
                if t == TBLOCKS[j][1] - 1 and EMIT_PROJ:
                    proj_block(j)

    nc.compile()
    return nc


def _shard_inputs(feat, W_hp, b_hp, W_ih, W_hh, b_ih, b_hh, embed, W_out, b_out):
    bf = ml_dtypes.bfloat16
    featT = np.ascontiguousarray(feat.T, dtype=np.float32)
    WhpT = np.ascontiguousarray(W_hp.T, dtype=np.float32)
    WihT = np.ascontiguousarray(W_ih.T).astype(bf)
    WhhT = np.ascontiguousarray(W_hh.T, dtype=np.float32)
    x0T = np.ascontiguousarray(
        np.repeat(np.asarray(embed)[SOS][:, None], BATCH, axis=1)
    ).astype(bf)
    Wo = np.zeros((NCORES * VPAD, HID), np.float32)
    Wo[:VOCAB] = W_out
    bo = np.zeros((NCORES * VPAD,), np.float32)
    bo[:VOCAB] = b_out
    common = dict(
        featT=featT, WhpT=WhpT, WihT=WihT, WhhT=WhhT,
        b_ih=np.asarray(b_ih, np.float32), b_hh=np.asarray(b_hh, np.float32),
        b_hp=np.asarray(b_hp, np.float32), x0T=x0T,
    )
    in_maps = []
    for c in range(NCORES):
        sl = slice(c * VPAD, (c + 1) * VPAD)
        m = dict(common)
        m["WoutT"] = np.ascontiguousarray(Wo[sl].T)
        m["b_out"] = bo[sl].copy()
        in_maps.append(m)
    return in_maps


def kernel(**inputs):
    global LAST_RESULTS
    args = {k: np.asarray(v) for k, v in inputs.items()}
    in_maps = _shard_inputs(
        args["feat"], args["W_hp"], args["b_hp"], args["W_ih"], args["W_hh"],
        args["b_ih"], args["b_hh"], args["embed"], args["W_out"], args["b_out"],
    )
    nc = build()
    res = run_bass_kernel_spmd(nc, in_maps, core_ids=list(range(NCORES)))
    LAST_RESULTS = res
    out = np.concatenate([r["OUT"] for r in res.results], axis=1)[:, :VOCAB, :]
    return np.ascontiguousarray(out, dtype=np.float32)
